# revision 24
# baseline (speedup 1.0000x reference)
"""Trainium2 Bass kernel for nn_CausalDAG (gnn_message_passing).

Computation (per batch row b):
    m[b]   = A^T @ x[b]                      # concept mixing, [C, D]
    h[b,c] = ELU(W1[c] @ m[b,c] + b1[c])     # per-concept Linear(D->G)
    out[b,c] = W2[c] @ h[b,c] + b2[c]        # per-concept Linear(G->D)

Kernel strategy (pure data-parallel over batch, 8 cores):
  - Fuse step 1+2 into one dense matmul: H_pre = X @ U where
    U[(j,d),(c,g)] = A[j,c] * W1[c,g,d]  ([1024, 512], dense).
  - x is pre-transposed on the host to [F, B_core] f16, so contraction
    slices DMA straight into SBUF as matmul rhs - no on-device transpose.
  - ELU is computed as h' = max(u+1, exp(u)) (= elu(u) + 1, since
    exp(u) >= u+1 with equality only at 0); the "+1" is folded into an
    adjusted output bias b2_eff = b2 - sum_g W2.
  - Step 3 uses h' tiles as the matmul *stationary* operand so the output
    lands directly in natural [batch, (c,d)] layout - no output transpose.

Self-contained: hardcodes shapes; only imports the system concourse repo.
"""

import os
import sys

import numpy as np

for _p in ("/opt/trn_rl_repo", "/root/.axon_site/_ro/trn_rl_repo"):
    if os.path.isdir(_p) and _p not in sys.path:
        sys.path.insert(0, _p)
        break

import concourse.bass as bass  # noqa: E402
import concourse.bacc as bacc_mod  # noqa: E402
import concourse.mybir as mybir  # noqa: E402
import concourse.tile as tile  # noqa: E402
from concourse.bass_utils import run_bass_kernel_spmd  # noqa: E402

try:
    import ml_dtypes

    _BF16_NP = ml_dtypes.bfloat16
except ImportError:  # pragma: no cover
    _BF16_NP = None

B, C, D, G = 65536, 16, 64, 32
F = C * D  # 1024 flattened feature dim
H = C * G  # 512 hidden dim
N_CORES = 8
B_CORE = B // N_CORES  # 8192
NB = 512  # batch rows per full chunk
KSL = F // 128  # 8 contraction slices
NQ = H // 128  # 4 concept groups of (4 concepts x 32 g)
# Chunk schedule (uniform; edge-shrinking measured slower on PE).
CHUNK_NB = [NB] * (B_CORE // NB)
assert sum(CHUNK_NB) == B_CORE
N_CHUNKS = len(CHUNK_NB)
CHUNK_B0 = [sum(CHUNK_NB[:i]) for i in range(N_CHUNKS)]
NT_MAX = NB // 128

MM_MODE = os.environ.get("GNN_MM_MODE", "f16")


def _mm_dtypes(mode):
    if mode == "bf16":
        return mybir.dt.bfloat16, np.dtype(_BF16_NP)
    if mode == "f16":
        return mybir.dt.float16, np.dtype(np.float16)
    if mode == "f32r":
        return mybir.dt.float32r, np.dtype(np.float32)
    return mybir.dt.float32, np.dtype(np.float32)


def build_bass(mode=MM_MODE):
    mm_dt, _ = _mm_dtypes(mode)
    f32 = mybir.dt.float32
    compact = mm_dt in (mybir.dt.bfloat16, mybir.dt.float16)
    o_dt = mm_dt if compact else f32

    nc = bacc_mod.Bacc()
    UQW = KSL * 128  # per-q stationary block width
    BW = F + 2 * NQ  # b2r | b1q | b1p1q  (f32)
    # x arrives pre-transposed: [F, B_CORE], f-major.
    xs = nc.declare_dram_parameter("xs", [F, B_CORE], mm_dt, isOutput=False)
    usb_d = [
        nc.declare_dram_parameter(f"usb{q}", [128, UQW], mm_dt, isOutput=False)
        for q in range(NQ)
    ]
    vsb_d = nc.declare_dram_parameter("vsb", [128, NQ * 256], mm_dt, isOutput=False)
    cstb_d = nc.declare_dram_parameter("cstb", [128, BW], f32, isOutput=False)
    out_d = nc.declare_dram_parameter("out", [B_CORE, F], o_dt, isOutput=True)

    with tile.TileContext(nc) as tc:
        with (
            tc.tile_pool(name="consts", bufs=1) as consts,
            tc.tile_pool(name="xt", bufs=4) as xt_pool,
            tc.tile_pool(name="hmat", bufs=2) as h_pool,
            tc.tile_pool(name="escr", bufs=6) as e_pool,
            tc.tile_pool(name="osb", bufs=2) as out_pool,
            tc.tile_pool(name="hp", bufs=4, space="PSUM") as hp_pool,
            tc.tile_pool(name="outp", bufs=4, space="PSUM") as outp_pool,
        ):
            # Load order: usb[0] -> first x chunk -> rest, so the first
            # step-1+2 matmul waits on 256KB, not the whole constant set.
            usb = [
                consts.tile([128, UQW], mm_dt, tag=f"usb{q}", name=f"usb{q}")
                for q in range(NQ)
            ]
            nc.sync.dma_start(usb[0][:], usb_d[0][:])

            def tail_piece(prev_hqs, o_t, t):
                # step3 + bias for one b-tile of a finished chunk; emitted
                # interleaved with the next chunk's fused matmuls so the
                # vector evictions overlap PE work instead of gating it.
                for half in range(2):
                    op = outp_pool.tile([128, 512], f32, tag="outp")
                    for qq in range(2):
                        q = half * 2 + qq
                        nc.tensor.matmul(
                            op[:, qq * 256 : (qq + 1) * 256],
                            lhsT=prev_hqs[q][:, t * 128 : (t + 1) * 128],
                            rhs=vsb[:, q * 256 : (q + 1) * 256],
                            start=True,
                            stop=True,
                        )
                    nc.vector.scalar_tensor_tensor(
                        o_t[:, t * F + half * 512 : t * F + (half + 1) * 512],
                        op[:],
                        1.0,
                        b2r[:, half * 512 : (half + 1) * 512],
                        mybir.AluOpType.mult,
                        mybir.AluOpType.add,
                    )

            def store_out(prev_ci, o_t):
                b0p, nt = CHUNK_B0[prev_ci], CHUNK_NB[prev_ci] // 128
                dstp = out_d[b0p : b0p + nt * 128, :].rearrange(
                    "(t p) f -> p t f", p=128
                )
                srcp = o_t[:, 0 : nt * F].rearrange("p (t f) -> p t f", t=nt)
                nc.scalar.dma_start(dstp, srcp)

            def load_chunk(ci):
                b0, nb = CHUNK_B0[ci], CHUNK_NB[ci]
                x_t = xt_pool.tile([128, KSL * NB], mm_dt, tag="xt")
                src = xs[:, b0 : b0 + nb].rearrange("(k p) b -> p k b", p=128)
                dst = x_t[:, 0 : KSL * nb].rearrange("p (k b) -> p k b", b=nb)
                nc.gpsimd.dma_start(dst, src)
                return x_t

            PREF = 4
            xq = [load_chunk(0), load_chunk(1)]
            for q in range(1, NQ):
                nc.sync.dma_start(usb[q][:], usb_d[q][:])
            vsb = consts.tile([128, NQ * 256], mm_dt, tag="vsb")
            nc.sync.dma_start(vsb[:], vsb_d[:])
            cstb = consts.tile([128, BW], f32, tag="cstb")
            nc.sync.dma_start(cstb[:], cstb_d[:])
            b2r = cstb[:, 0:F]
            b1q = cstb[:, F : F + NQ]
            xq += [load_chunk(ci) for ci in range(2, min(PREF, N_CHUNKS))]

            prev = None
            for ci in range(N_CHUNKS):
                if ci + PREF < N_CHUNKS:
                    xq.append(load_chunk(ci + PREF))
                x_t = xq.pop(0)
                nb = CHUNK_NB[ci]

                if prev is not None:
                    o_t = out_pool.tile([128, NT_MAX * F], o_dt, tag="osb")
                    nt_prev = CHUNK_NB[prev[0]] // 128

                # ---- fused step 1+2: H_T[q] = U_q^T @ X_T  (PSUM f32) ----
                hps = []
                for q in range(NQ):
                    # ---- step 3 b-tiles of the PREVIOUS chunk ----
                    if prev is not None and q < nt_prev:
                        tail_piece(prev[1], o_t, q)
                    hp = hp_pool.tile([128, NB], f32, tag="hp")
                    for k in range(KSL):
                        nc.tensor.matmul(
                            hp[:, 0:nb],
                            lhsT=usb[q][:, k * 128 : (k + 1) * 128],
                            rhs=x_t[:, k * nb : (k + 1) * nb],
                            start=(k == 0),
                            stop=(k == KSL - 1),
                        )
                    # ---- ELU': h' = relu(u) + min(exp(u), 1), u = z + b1 ----
                    e_t = e_pool.tile([128, NB], mm_dt, tag="ee")
                    nc.scalar.activation(
                        e_t[:, 0:nb],
                        hp[:, 0:nb],
                        mybir.ActivationFunctionType.Exp,
                        bias=b1q[:, q : q + 1],
                        scale=1.0,
                    )
                    r_t = e_pool.tile([128, NB], mm_dt, tag="er")
                    nc.scalar.activation(
                        r_t[:, 0:nb],
                        hp[:, 0:nb],
                        mybir.ActivationFunctionType.Relu,
                        bias=b1q[:, q : q + 1],
                        scale=1.0,
                    )
                    hps.append((e_t, r_t))

                if prev is not None:
                    store_out(prev[0], o_t)

                # h'-evictions deferred behind the tail bias-adds on the
                # vector queue so the step-3 matmuls are never gated on them.
                hqs = []
                for q in range(NQ):
                    e_t, r_t = hps[q]
                    h_q = h_pool.tile([128, NB], mm_dt, tag=f"h{q}")
                    nc.vector.scalar_tensor_tensor(
                        h_q[:, 0:nb],
                        e_t[:, 0:nb],
                        1.0,
                        r_t[:, 0:nb],
                        mybir.AluOpType.min,
                        mybir.AluOpType.add,
                    )
                    hqs.append(h_q)
                prev = (ci, hqs)

            # final chunk drain: store per b-tile so DMA overlaps eviction
            o_t = out_pool.tile([128, NT_MAX * F], o_dt, tag="osb")
            b0p = CHUNK_B0[prev[0]]
            for t in range(CHUNK_NB[prev[0]] // 128):
                tail_piece(prev[1], o_t, t)
                dstp = out_d[b0p + t * 128 : b0p + (t + 1) * 128, :]
                nc.scalar.dma_start(dstp, o_t[:, t * F : (t + 1) * F])

    nc.compile()
    return nc


def _host_tensors(A, W1, b1, W2, b2, mode=MM_MODE):
    _, mm_np = _mm_dtypes(mode)
    A = np.asarray(A, np.float32)
    W1 = np.asarray(W1, np.float32)
    b1 = np.asarray(b1, np.float32)
    W2 = np.asarray(W2, np.float32)
    b2 = np.asarray(b2, np.float32)

    # U[(j,d), (c,g)] = A[j,c] * W1[c,g,d]
    U = np.einsum("jc,cgd->jdcg", A, W1).reshape(F, H)
    # usb[p, k*H + m] = U[k*128 + p, m]
    usb = np.ascontiguousarray(U.reshape(KSL, 128, H).transpose(1, 0, 2).reshape(128, KSL * H))
    # V_q[(ct,g), (ct',d)] = delta * W2[4q+ct, d, g]; vsb[p, q*256 + n]
    vsb = np.zeros((128, NQ * 256), np.float32)
    for q in range(NQ):
        for ct in range(4):
            c = 4 * q + ct
            vsb[ct * G : (ct + 1) * G, q * 256 + ct * D : q * 256 + (ct + 1) * D] = W2[c].T
    b1cols = b1.reshape(H)  # [(c,g)] c-major == (q, ct, g)
    b1q = b1cols.reshape(NQ, 128).T  # [128, NQ]
    b2eff = (b2 - W2.sum(axis=2)).reshape(F)
    b2r = np.broadcast_to(b2eff, (128, F))
    cstb = np.concatenate(
        [np.asarray(b2r, np.float32), b1q, b1q + 1.0], axis=1
    ).astype(np.float32)
    out = {
        "vsb": np.ascontiguousarray(vsb.astype(mm_np)),
        "cstb": np.ascontiguousarray(cstb),
    }
    usb3 = usb.reshape(128, KSL, H)
    for q in range(NQ):
        out[f"usb{q}"] = np.ascontiguousarray(
            usb3[:, :, q * 128 : (q + 1) * 128].reshape(128, KSL * 128).astype(mm_np)
        )
    return out


def kernel(x, A, W1, b1, W2, b2, mode=MM_MODE, trace=False):
    _, mm_np = _mm_dtypes(mode)
    # Pre-transpose on host: xs_all[f, b] so contraction slices DMA straight
    # to SBUF in matmul layout (layout-only transform, no arithmetic on x).
    x = np.asarray(x, np.float32).reshape(B, F).astype(mm_np, copy=False)
    xs_all = np.ascontiguousarray(x.T)  # [F, B]
    weights = _host_tensors(A, W1, b1, W2, b2, mode)

    nc = build_bass(mode)
    in_maps = []
    for i in range(N_CORES):
        m = {"xs": np.ascontiguousarray(xs_all[:, i * B_CORE : (i + 1) * B_CORE])}
        m.update(weights)
        in_maps.append(m)

    res = run_bass_kernel_spmd(nc, in_maps, core_ids=list(range(N_CORES)), trace=trace)
    out = np.concatenate([r["out"] for r in res.results], axis=0)
    out = out.reshape(B, C, D).astype(np.float32)
    if trace:
        return out, res
    return out


# revision 26
# speedup vs baseline: 1.0597x; 1.0597x over previous
"""Trainium2 Bass kernel for nn_CausalDAG (gnn_message_passing).

Computation (per batch row b):
    m[b]   = A^T @ x[b]                      # concept mixing, [C, D]
    h[b,c] = ELU(W1[c] @ m[b,c] + b1[c])     # per-concept Linear(D->G)
    out[b,c] = W2[c] @ h[b,c] + b2[c]        # per-concept Linear(G->D)

Kernel strategy (pure data-parallel over batch, 8 cores):
  - Fuse step 1+2 into one dense matmul: H_pre = X @ U where
    U[(j,d),(c,g)] = A[j,c] * W1[c,g,d]  ([1024, 512], dense).
  - x is pre-transposed on the host to [F, B_core] f16, so contraction
    slices DMA straight into SBUF as matmul rhs - no on-device transpose.
  - ELU is computed as h' = max(u+1, exp(u)) (= elu(u) + 1, since
    exp(u) >= u+1 with equality only at 0); the "+1" is folded into an
    adjusted output bias b2_eff = b2 - sum_g W2.
  - Step 3 uses h' tiles as the matmul *stationary* operand so the output
    lands directly in natural [batch, (c,d)] layout - no output transpose.

Self-contained: hardcodes shapes; only imports the system concourse repo.
"""

import os
import sys

import numpy as np

for _p in ("/opt/trn_rl_repo", "/root/.axon_site/_ro/trn_rl_repo"):
    if os.path.isdir(_p) and _p not in sys.path:
        sys.path.insert(0, _p)
        break

import concourse.bass as bass  # noqa: E402
import concourse.bacc as bacc_mod  # noqa: E402
import concourse.mybir as mybir  # noqa: E402
import concourse.tile as tile  # noqa: E402
from concourse.bass_utils import run_bass_kernel_spmd  # noqa: E402

try:
    import ml_dtypes

    _BF16_NP = ml_dtypes.bfloat16
except ImportError:  # pragma: no cover
    _BF16_NP = None

B, C, D, G = 65536, 16, 64, 32
F = C * D  # 1024 flattened feature dim
H = C * G  # 512 hidden dim
N_CORES = 8
B_CORE = B // N_CORES  # 8192
NB = 512  # batch rows per full chunk
KSL = F // 128  # 8 contraction slices
NQ = H // 128  # 4 concept groups of (4 concepts x 32 g)
# Chunk schedule (uniform; edge-shrinking measured slower on PE).
CHUNK_NB = [NB] * (B_CORE // NB)
assert sum(CHUNK_NB) == B_CORE
N_CHUNKS = len(CHUNK_NB)
CHUNK_B0 = [sum(CHUNK_NB[:i]) for i in range(N_CHUNKS)]
NT_MAX = NB // 128

MM_MODE = os.environ.get("GNN_MM_MODE", "f16")


def _mm_dtypes(mode):
    if mode == "bf16":
        return mybir.dt.bfloat16, np.dtype(_BF16_NP)
    if mode == "f16":
        return mybir.dt.float16, np.dtype(np.float16)
    if mode == "f32r":
        return mybir.dt.float32r, np.dtype(np.float32)
    return mybir.dt.float32, np.dtype(np.float32)


def build_bass(mode=MM_MODE):
    mm_dt, _ = _mm_dtypes(mode)
    f32 = mybir.dt.float32
    compact = mm_dt in (mybir.dt.bfloat16, mybir.dt.float16)
    o_dt = mm_dt if compact else f32

    nc = bacc_mod.Bacc()
    UQW = KSL * 128  # per-q stationary block width
    BW = F + 2 * NQ  # b2r | b1q | b1p1q  (f32)
    # x arrives pre-transposed: [F, B_CORE], f-major.
    xs = nc.declare_dram_parameter("xs", [F, B_CORE], mm_dt, isOutput=False)
    usb_d = [
        nc.declare_dram_parameter(f"usb{q}", [128, UQW], mm_dt, isOutput=False)
        for q in range(NQ)
    ]
    vsb_d = nc.declare_dram_parameter("vsb", [128, NQ * 256], mm_dt, isOutput=False)
    cstb_d = nc.declare_dram_parameter("cstb", [128, BW], f32, isOutput=False)
    out_d = nc.declare_dram_parameter("out", [B_CORE, F], o_dt, isOutput=True)

    with tile.TileContext(nc) as tc:
        with (
            tc.tile_pool(name="consts", bufs=1) as consts,
            tc.tile_pool(name="xt", bufs=4) as xt_pool,
            tc.tile_pool(name="hmat", bufs=2) as h_pool,
            tc.tile_pool(name="escr", bufs=6) as e_pool,
            tc.tile_pool(name="osb", bufs=2) as out_pool,
            tc.tile_pool(name="hp", bufs=4, space="PSUM") as hp_pool,
            tc.tile_pool(name="outp", bufs=4, space="PSUM") as outp_pool,
        ):
            # Load order: usb[0] -> first x chunk -> rest, so the first
            # step-1+2 matmul waits on 256KB, not the whole constant set.
            usb = [
                consts.tile([128, UQW], mm_dt, tag=f"usb{q}", name=f"usb{q}")
                for q in range(NQ)
            ]
            nc.sync.dma_start(usb[0][:], usb_d[0][:])

            def tail_piece(prev_hqs, o_t, t):
                # step3 + bias for one b-tile of a finished chunk; emitted
                # interleaved with the next chunk's fused matmuls so the
                # vector evictions overlap PE work instead of gating it.
                for half in range(2):
                    op = outp_pool.tile([128, 512], f32, tag="outp")
                    for qq in range(2):
                        q = half * 2 + qq
                        nc.tensor.matmul(
                            op[:, qq * 256 : (qq + 1) * 256],
                            lhsT=prev_hqs[q][:, t * 128 : (t + 1) * 128],
                            rhs=vsb[:, q * 256 : (q + 1) * 256],
                            start=True,
                            stop=True,
                        )
                    nc.vector.scalar_tensor_tensor(
                        o_t[:, t * F + half * 512 : t * F + (half + 1) * 512],
                        op[:],
                        1.0,
                        b2r[:, half * 512 : (half + 1) * 512],
                        mybir.AluOpType.mult,
                        mybir.AluOpType.add,
                    )

            def store_out(prev_ci, o_t):
                b0p, nt = CHUNK_B0[prev_ci], CHUNK_NB[prev_ci] // 128
                dstp = out_d[b0p : b0p + nt * 128, :].rearrange(
                    "(t p) f -> p t f", p=128
                )
                srcp = o_t[:, 0 : nt * F].rearrange("p (t f) -> p t f", t=nt)
                nc.scalar.dma_start(dstp, srcp)

            def load_chunk(ci):
                b0, nb = CHUNK_B0[ci], CHUNK_NB[ci]
                x_t = xt_pool.tile([128, KSL * NB], mm_dt, tag="xt")
                src = xs[:, b0 : b0 + nb].rearrange("(k p) b -> p k b", p=128)
                dst = x_t[:, 0 : KSL * nb].rearrange("p (k b) -> p k b", b=nb)
                nc.sync.dma_start(dst, src)
                return x_t

            PREF = 3
            xq = [load_chunk(0)]
            for q in range(1, NQ):
                nc.sync.dma_start(usb[q][:], usb_d[q][:])
            vsb = consts.tile([128, NQ * 256], mm_dt, tag="vsb")
            nc.sync.dma_start(vsb[:], vsb_d[:])
            cstb = consts.tile([128, BW], f32, tag="cstb")
            nc.sync.dma_start(cstb[:], cstb_d[:])
            b2r = cstb[:, 0:F]
            b1q = cstb[:, F : F + NQ]
            xq += [load_chunk(ci) for ci in range(1, min(PREF, N_CHUNKS))]

            prev = None
            for ci in range(N_CHUNKS):
                if ci + PREF < N_CHUNKS:
                    xq.append(load_chunk(ci + PREF))
                x_t = xq.pop(0)
                nb = CHUNK_NB[ci]

                if prev is not None:
                    o_t = out_pool.tile([128, NT_MAX * F], o_dt, tag="osb")
                    nt_prev = CHUNK_NB[prev[0]] // 128

                # ---- fused step 1+2: H_T[q] = U_q^T @ X_T  (PSUM f32) ----
                hps = []
                for q in range(NQ):
                    # ---- step 3 b-tiles of the PREVIOUS chunk ----
                    if prev is not None and q < nt_prev:
                        tail_piece(prev[1], o_t, q)
                    hp = hp_pool.tile([128, NB], f32, tag="hp")
                    for k in range(KSL):
                        nc.tensor.matmul(
                            hp[:, 0:nb],
                            lhsT=usb[q][:, k * 128 : (k + 1) * 128],
                            rhs=x_t[:, k * nb : (k + 1) * nb],
                            start=(k == 0),
                            stop=(k == KSL - 1),
                        )
                    # ---- ELU': h' = relu(u) + min(exp(u), 1), u = z + b1 ----
                    e_t = e_pool.tile([128, NB], mm_dt, tag="ee")
                    nc.scalar.activation(
                        e_t[:, 0:nb],
                        hp[:, 0:nb],
                        mybir.ActivationFunctionType.Exp,
                        bias=b1q[:, q : q + 1],
                        scale=1.0,
                    )
                    r_t = e_pool.tile([128, NB], mm_dt, tag="er")
                    nc.scalar.activation(
                        r_t[:, 0:nb],
                        hp[:, 0:nb],
                        mybir.ActivationFunctionType.Relu,
                        bias=b1q[:, q : q + 1],
                        scale=1.0,
                    )
                    hps.append((e_t, r_t))

                if prev is not None:
                    store_out(prev[0], o_t)

                # h'-evictions deferred behind the tail bias-adds on the
                # vector queue so the step-3 matmuls are never gated on them.
                hqs = []
                for q in range(NQ):
                    e_t, r_t = hps[q]
                    h_q = h_pool.tile([128, NB], mm_dt, tag=f"h{q}")
                    nc.vector.scalar_tensor_tensor(
                        h_q[:, 0:nb],
                        e_t[:, 0:nb],
                        1.0,
                        r_t[:, 0:nb],
                        mybir.AluOpType.min,
                        mybir.AluOpType.add,
                    )
                    hqs.append(h_q)
                prev = (ci, hqs)

            # final chunk drain: store per b-tile so DMA overlaps eviction
            o_t = out_pool.tile([128, NT_MAX * F], o_dt, tag="osb")
            b0p = CHUNK_B0[prev[0]]
            for t in range(CHUNK_NB[prev[0]] // 128):
                tail_piece(prev[1], o_t, t)
                dstp = out_d[b0p + t * 128 : b0p + (t + 1) * 128, :]
                nc.scalar.dma_start(dstp, o_t[:, t * F : (t + 1) * F])

    nc.compile()
    return nc


def _host_tensors(A, W1, b1, W2, b2, mode=MM_MODE):
    _, mm_np = _mm_dtypes(mode)
    A = np.asarray(A, np.float32)
    W1 = np.asarray(W1, np.float32)
    b1 = np.asarray(b1, np.float32)
    W2 = np.asarray(W2, np.float32)
    b2 = np.asarray(b2, np.float32)

    # U[(j,d), (c,g)] = A[j,c] * W1[c,g,d]
    U = np.einsum("jc,cgd->jdcg", A, W1).reshape(F, H)
    # usb[p, k*H + m] = U[k*128 + p, m]
    usb = np.ascontiguousarray(U.reshape(KSL, 128, H).transpose(1, 0, 2).reshape(128, KSL * H))
    # V_q[(ct,g), (ct',d)] = delta * W2[4q+ct, d, g]; vsb[p, q*256 + n]
    vsb = np.zeros((128, NQ * 256), np.float32)
    for q in range(NQ):
        for ct in range(4):
            c = 4 * q + ct
            vsb[ct * G : (ct + 1) * G, q * 256 + ct * D : q * 256 + (ct + 1) * D] = W2[c].T
    b1cols = b1.reshape(H)  # [(c,g)] c-major == (q, ct, g)
    b1q = b1cols.reshape(NQ, 128).T  # [128, NQ]
    b2eff = (b2 - W2.sum(axis=2)).reshape(F)
    b2r = np.broadcast_to(b2eff, (128, F))
    cstb = np.concatenate(
        [np.asarray(b2r, np.float32), b1q, b1q + 1.0], axis=1
    ).astype(np.float32)
    out = {
        "vsb": np.ascontiguousarray(vsb.astype(mm_np)),
        "cstb": np.ascontiguousarray(cstb),
    }
    usb3 = usb.reshape(128, KSL, H)
    for q in range(NQ):
        out[f"usb{q}"] = np.ascontiguousarray(
            usb3[:, :, q * 128 : (q + 1) * 128].reshape(128, KSL * 128).astype(mm_np)
        )
    return out


def kernel(x, A, W1, b1, W2, b2, mode=MM_MODE, trace=False):
    _, mm_np = _mm_dtypes(mode)
    # Pre-transpose on host: xs_all[f, b] so contraction slices DMA straight
    # to SBUF in matmul layout (layout-only transform, no arithmetic on x).
    x = np.asarray(x, np.float32).reshape(B, F).astype(mm_np, copy=False)
    xs_all = np.ascontiguousarray(x.T)  # [F, B]
    weights = _host_tensors(A, W1, b1, W2, b2, mode)

    nc = build_bass(mode)
    in_maps = []
    for i in range(N_CORES):
        m = {"xs": np.ascontiguousarray(xs_all[:, i * B_CORE : (i + 1) * B_CORE])}
        m.update(weights)
        in_maps.append(m)

    res = run_bass_kernel_spmd(nc, in_maps, core_ids=list(range(N_CORES)), trace=trace)
    out = np.concatenate([r["out"] for r in res.results], axis=0)
    out = out.reshape(B, C, D).astype(np.float32)
    if trace:
        return out, res
    return out
